# revision 20
# baseline (speedup 1.0000x reference)
"""Llama GQA attention layer (B=1, S=2048, E=4096, H=32, HKV=8, D=128) on 8
Trainium2 NeuronCores.

Sharding: tensor-parallel over heads. Core c owns Q heads 4c..4c+3 and KV head
c (KV groups stay intact), plus the matching Wo input-dim slice. Each core
computes a full [S, E] partial of the o_proj output in bf16; the host sums the
8 partials (the "all-reduce after o_proj").

All matmuls run in bf16 (1 cyc/row on the PE at 512-wide moving dim, with
automatic fast-weight-load; fp32r streams at ~1.3 cyc/row and pays 225ns
weight loads). PSUM accumulation stays fp32. l2 error budget is 2e-2; bf16
rounding of inputs/weights/probabilities lands well under 1e-2.

Per-core dataflow:
  phase A (PE-dense): per token group g (512 tokens):
    qT/kT/vT = W @ hs.T    6 psum chains x 32 E-chunks, [feat, tok] layout.
    Wq|Wk|Wv are host-packed into one [E, 768] tensor so each E-chunk is a
    single contiguous DMA and the first matmul fires ~2us after launch.
    RoPE off-PE: psum -> sbuf copy (scalar), half-swap via SBUF->SBUF DMA,
    cos/sin muls (DVE) -> qro/krope bf16. v: psum -> bf16 sbuf (scalar),
    PE-transposed to vnat [tok, d] between later groups' QKV streams.
  phase B attention, per query group G (causal: key tiles ki <= 4G+3, with
  moving-dim trimming + triangular mask add on diagonal tiles):
    scoresT[k, q] = krope_tile^T @ qro  (PSUM), exp on ScalarE -> bf16 expT
    avT[d, q]  accumulated over ki on PE (vnat stationary)
    den[h]     accumulated over ki on PE (ones[128,1] stationary, ~free
               weight load); all 4 heads share one psum bank at partition
               offsets 0/32/64/96.
    epilogue per head: DVE reciprocal on the [1, 512] den row only (full-tile
    reciprocal costs ~12 cyc/elem), K=1 ones matmul broadcasts it to 128
    partitions, DVE mul -> aoT bf16 (kept in SBUF, no DRAM spill).
  phase C o_proj: out[t, e] = sum_h aoT[:, h-tile]^T @ woT[h], 3 psum
    banks rotating, drains alternate scalar/vector, bf16 partials to DRAM.
"""

import sys
import types

if "/opt/trn_rl_repo" not in sys.path:
    sys.path.insert(0, "/opt/trn_rl_repo")

import numpy as np
import ml_dtypes

import concourse.bass as bass
import concourse.tile as tile
from concourse import bacc, mybir
from concourse.bass_utils import run_bass_kernel_spmd
from concourse.masks import make_identity

F32 = mybir.dt.float32
BF16 = mybir.dt.bfloat16
EXP = mybir.ActivationFunctionType.Exp
NPBF = ml_dtypes.bfloat16

S = 2048
E = 4096
H = 32
HKV = 8
D = 128
NCORES = 8
HL = H // NCORES          # 4 local q heads per core
TG = 512                  # token group (moving-dim tile)
NG = S // TG              # 4 token groups
NE = E // 128             # 32 contraction chunks
NK = S // 128             # 16 key tiles
FQKV = HL * D + 2 * D     # 768 packed output features per core
NEG = -1e9

TRACE = [False]
LAST_EXEC_NS = [None]
LAST_RES = [None]

_PROGRAMS = {}


def _install_ntff_hook():
    if "antenv.axon_hooks" in sys.modules:
        return
    mod = types.ModuleType("antenv.axon_hooks")
    hook = [None]
    mod.set_axon_ntff_profile_hook = lambda h: hook.__setitem__(0, h)
    mod.get_axon_ntff_profile_hook = lambda: hook[0]
    sys.modules["antenv.axon_hooks"] = mod
    try:
        from trn_agent_boot.trn_boot import _ntff_profile_via_ctypes

        mod.set_axon_ntff_profile_hook(
            _ntff_profile_via_ctypes("/opt/axon/libaxon_pjrt.so"))
    except Exception:
        pass


def set_trace(on=True):
    if on:
        _install_ntff_hook()
    TRACE[0] = on


def _build_program(mode):
    """mode: 'causal' (skip above-diagonal key tiles, trim + triangular mask
    on diagonal tiles), 'full' (no mask), 'general' (additive mask streamed
    from DRAM)."""
    nc = bacc.Bacc(trn_type="TRN2", target_bir_lowering=False, debug=False)

    # group-major hsT: [g, E, TG] so each [128, TG] chunk is contiguous
    hsT_d = nc.dram_tensor("hsT", [NG, E, TG], BF16, kind="ExternalInput").ap()
    # packed [Wq | Wk | Wv] transposed: rows are E, cols 768
    wqkv_d = nc.dram_tensor("wqkvT", [E, FQKV], BF16, kind="ExternalInput").ap()
    woT_d = nc.dram_tensor("woT", [HL * D, E], BF16, kind="ExternalInput").ap()
    cos_d = nc.dram_tensor("cosT", [D, S], F32, kind="ExternalInput").ap()
    sin_d = nc.dram_tensor("sinT", [D, S], F32, kind="ExternalInput").ap()
    if mode == "causal":
        cmask_d = nc.dram_tensor("cmask", [128, 128], F32,
                                 kind="ExternalInput").ap()
    elif mode == "general":
        maskT_d = nc.dram_tensor("maskT", [S, S], F32, kind="ExternalInput").ap()
    # tile-major output: [ti, eg, 128, TG] so each store is contiguous
    outp_d = nc.dram_tensor("outp", [NK, E // TG, 128, TG], BF16,
                            kind="ExternalOutput").ap()

    with tile.TileContext(nc) as tc:
        with tc.tile_pool(name="const", bufs=1) as cpool, \
             tc.tile_pool(name="persist", bufs=1) as pp, \
             tc.tile_pool(name="wqkv", bufs=1) as wp, \
             tc.tile_pool(name="cs", bufs=1) as csp, \
             tc.tile_pool(name="hst", bufs=8) as hp, \
             tc.tile_pool(name="rope", bufs=1) as rp, \
             tc.tile_pool(name="attn", bufs=1) as ap_, \
             tc.tile_pool(name="outb", bufs=1) as obp, \
             tc.tile_pool(name="ps", bufs=1, space="PSUM") as ps:

            # ---- constants ----
            identf = cpool.tile([128, 128], F32)
            make_identity(nc, identf)
            ident = cpool.tile([128, 128], BF16)
            nc.vector.tensor_copy(ident, identf)
            onesf = cpool.tile([128, 32], F32)
            nc.vector.memset(onesf, 1.0)
            ones_col = cpool.tile([128, 32], BF16)
            nc.vector.tensor_copy(ones_col, onesf)
            # ones rows at partition bases 0 and 32 (matmul operands must
            # share a 32-aligned base partition with the den rows they read)
            onesrf = cpool.tile([64, 128], F32)
            nc.vector.memset(onesrf, 1.0)
            ones_rows = cpool.tile([64, 128], BF16)
            nc.vector.tensor_copy(ones_rows, onesrf)
            if mode == "causal":
                cmask = cpool.tile([128, 128], F32)

            # ---- persistent activations ----
            krope = pp.tile([128, S], BF16)               # [d, tok]
            vnat = pp.tile([128, NK, 128], BF16)          # [tok%128, ktile, d]
            ao = pp.tile([128, HL, S], BF16)              # [d, head, tok]
            qro = pp.tile([128, NG, HL, TG], BF16)        # [d, g, head, tok]

            # ---- weights: chunk loads interleaved with group-0 hsT so the
            # first QKV matmul fires almost immediately ----
            w_sb = wp.tile([128, NE, FQKV], BF16)
            wo_sb = wp.tile([128, HL, E], BF16)
            cos_sb = csp.tile([128, S], F32)
            sin_sb = csp.tile([128, S], F32)

            # weight chunks batched x4 on the scalar queue, group-0 hsT on the
            # sync queue: both pipelines issue in parallel so the first
            # matmul fires as early as possible (each dma_start costs ~0.6us
            # of descriptor generation on its issuing queue)
            wqkv_r = wqkv_d.rearrange("(ne p) f -> p ne f", p=128)
            hst0 = []
            for e in range(NE):
                if e % 4 == 0:
                    nc.scalar.dma_start(out=w_sb[:, e:e + 4, :],
                                        in_=wqkv_r[:, e:e + 4, :])
                hst = hp.tile([128, TG], BF16, tag="hst")
                nc.sync.dma_start(
                    out=hst, in_=hsT_d[0, 128 * e:128 * (e + 1), :])
                hst0.append(hst)
            # cos/sin (2MB) + cmask load after the weights; not needed
            # until rope(0)/attention
            nc.scalar.dma_start(out=cos_sb, in_=cos_d)
            nc.scalar.dma_start(out=sin_sb, in_=sin_d)
            if mode == "causal":
                nc.scalar.dma_start(out=cmask, in_=cmask_d)

            # ================= phase A: QKV projection + RoPE =================
            def emit_qkv(g):
                q_ps = [ps.tile([128, TG], F32, tag=f"A{f}", name=f"q_ps{f}")
                        for f in range(HL)]
                k_ps = ps.tile([128, TG], F32, tag="A4", name="k_ps")
                v_ps = ps.tile([128, TG], F32, tag="A5", name="v_ps")
                for e in range(NE):
                    if g == 0:
                        hst = hst0[e]
                    else:
                        hst = hp.tile([128, TG], BF16, tag="hst")
                        nc.sync.dma_start(
                            out=hst, in_=hsT_d[g, 128 * e:128 * (e + 1), :])
                    st, sp = (e == 0), (e == NE - 1)
                    for f in range(HL):
                        nc.tensor.matmul(
                            q_ps[f], w_sb[:, e, 128 * f:128 * (f + 1)],
                            hst, start=st, stop=sp)
                    nc.tensor.matmul(k_ps, w_sb[:, e, 512:640], hst,
                                     start=st, stop=sp)
                    nc.tensor.matmul(v_ps, w_sb[:, e, 640:768], hst,
                                     start=st, stop=sp)
                return q_ps, k_ps, v_ps

            def emit_rope(g, q_ps, k_ps, v_ps):
                t0 = g * TG
                cs = cos_sb[:, t0:t0 + TG]
                sn = sin_sb[:, t0:t0 + TG]
                pairs = [(q_ps[f], qro[:, g, f, :]) for f in range(HL)]
                kpair = (k_ps, krope[:, t0:t0 + TG])
                # last group: drain k first so attention's first score matmul
                # (s-bank reuse of the k psum tag) unblocks earliest
                pairs = [kpair] + pairs if g == NG - 1 else pairs + [kpair]
                # drain ALL six psum banks first, copies split across scalar
                # and vector so the next group's QKV matmuls unblock in ~1us;
                # the rope math below then reads only SBUF
                vs = rp.tile([128, TG], BF16, tag="vs", bufs=2)
                if g == NG - 1:
                    nc.scalar.copy(out=vs, in_=v_ps)
                xss = []
                for i, (x_ps, _) in enumerate(pairs):
                    xs = rp.tile([128, TG], F32, tag="xs", bufs=6)
                    # last group: keep the scalar queue clear for attention's
                    # exps -- only k drains there, the q heads go to vector
                    if (i == 0) if g == NG - 1 else (i % 2 == 0):
                        nc.scalar.copy(out=xs, in_=x_ps)
                    else:
                        nc.vector.tensor_copy(xs, x_ps)
                    xss.append(xs)
                if g != NG - 1:
                    nc.scalar.copy(out=vs, in_=v_ps)

                def math():
                    for xs, (_, out_ap) in zip(xss, pairs):
                        swp = rp.tile([128, TG], F32, tag="swp", bufs=3)
                        nc.gpsimd.dma_start(out=swp[0:64, :], in_=xs[64:128, :])
                        nc.gpsimd.dma_start(out=swp[64:128, :], in_=xs[0:64, :])
                        p1 = rp.tile([128, TG], F32, tag="p1", bufs=2)
                        nc.vector.tensor_mul(p1, xs, cs)
                        nc.vector.tensor_mul(swp, swp, sn)
                        nc.vector.tensor_add(out_ap, p1, swp)
                # group 3's rope outputs are only read by attention(3): defer
                # its math emission behind attention(0) so group 0's cmask
                # adds and exps aren't queued behind it
                if g == NG - 1:
                    return vs, math
                math()
                return vs, None

            def emit_vtr(g, vs):
                for j in range(4):
                    tr = ps.tile([128, 128], BF16, tag="A6", name="tr_ps")
                    nc.tensor.transpose(tr, vs[:, 128 * j:128 * (j + 1)], ident)
                    nc.vector.tensor_copy(vnat[:, 4 * g + j, :], tr)

            vs_pend = []
            rope3_math = [None]
            for g in range(NG):
                qkv = emit_qkv(g)
                if vs_pend:
                    emit_vtr(*vs_pend.pop())
                vs, m = emit_rope(g, *qkv)
                rope3_math[0] = m
                vs_pend.append((g, vs))
            emit_vtr(*vs_pend.pop())

            # wo loads issue from the (otherwise idle) gpsimd queue during
            # attention
            woT_r = woT_d.rearrange("(h p) e -> p h e", p=128)
            for eg in range(E // TG):
                nc.gpsimd.dma_start(
                    out=wo_sb[:, :, TG * eg:TG * (eg + 1)],
                    in_=woT_r[:, :, TG * eg:TG * (eg + 1)])

            # ================= phase B: attention =================
            # Head-major: each head's full key sweep completes before the next
            # head begins, so the per-head softmax epilogue (3.3us flat DVE
            # reciprocal) overlaps the NEXT head's matmul stream instead of
            # four reciprocals serializing at the group boundary. Banks:
            # av alternates A0/A1, scores alternate A4/A5, each head owns a
            # private den bank (A2/A3/A6/A7) whose tag is reused for its own
            # broadcast matmul. Epilogues defer by at most one head: the
            # flush invariant keeps every bank's previous reader emitted
            # before its next writer (PE-queue deadlock freedom).
            deferred = []

            def emit_attn(G):
                nk = 4 * G + 4 if mode == "causal" else NK
                t0 = G * TG
                for h in range(HL):
                    while len(deferred) > 1:
                        deferred.pop(0)()
                    av = ps.tile([128, TG], F32, tag=["A0", "A1"][h % 2],
                                 name=f"av{h}")
                    den = ps.tile([128, TG], F32, name="den",
                                  tag=["A2", "A3", "A6", "A7"][h])
                    pend = []

                    def drain(item, av=av, den=den, nk=nk):
                        ki, c0, ex = item
                        nc.tensor.matmul(den[0:32, c0:], ones_col,
                                         ex[:, c0:],
                                         start=(ki == 0), stop=(ki == nk - 1),
                                         skip_group_check=True)
                        nc.tensor.matmul(av[:, c0:], vnat[:, ki, :],
                                         ex[:, c0:], start=(ki == 0),
                                         stop=(ki == nk - 1),
                                         skip_group_check=True)

                    for ki in range(nk):
                        c0 = max(0, 128 * ki - TG * G) if mode == "causal" else 0
                        s = ps.tile([128, TG], F32, name="s_ps",
                                    tag=["A4", "A5"][ki % 2])
                        nc.tensor.matmul(s[:, c0:],
                                         krope[:, 128 * ki:128 * (ki + 1)],
                                         qro[:, G, h, c0:],
                                         start=True, stop=True)
                        if mode == "causal" and ki >= 4 * G:
                            nc.vector.tensor_add(s[:, c0:c0 + 128],
                                                 s[:, c0:c0 + 128], cmask)
                        elif mode == "general":
                            mt = ap_.tile([128, TG], F32, tag="mt", bufs=4)
                            nc.sync.dma_start(
                                out=mt, in_=maskT_d[128 * ki:128 * (ki + 1),
                                                    TG * G:TG * (G + 1)])
                            nc.vector.tensor_add(s, s, mt)
                        ex = ap_.tile([128, TG], BF16, tag="ex", bufs=8)
                        nc.scalar.activation(out=ex[:, c0:], in_=s[:, c0:],
                                             func=EXP)
                        pend.append((ki, c0, ex))
                        # consume the previous head's epilogue only once its
                        # reciprocal has had a few microseconds of cover
                        if ki >= 6 and deferred:
                            deferred.pop(0)()
                        while len(pend) > 2:
                            drain(pend.pop(0))
                    while pend:
                        drain(pend.pop(0))
                    # reciprocal fires now, hidden under the next head's
                    # matmuls; the psum-side epilogue is deferred
                    rcb = ap_.tile([64, TG], BF16, tag="rcb", bufs=3)
                    with nc.allow_low_precision(reason="softmax recip"):
                        nc.vector.reciprocal(rcb[0:1, :], den[0:1, :])
                    bc = ps.tile([128, TG], F32, name="bc_ps",
                                 tag=["A2", "A3", "A6", "A7"][h])

                    def mk(h=h, rcb=rcb, bc=bc, avh=av, t0=t0):
                        def emit_epi():
                            nc.tensor.matmul(bc, ones_rows[0:1, :],
                                             rcb[0:1, :], start=True,
                                             stop=True)
                            bcs = ap_.tile([128, TG], BF16, tag="bcs", bufs=2)
                            nc.vector.tensor_copy(bcs, bc)
                            nc.vector.tensor_mul(ao[:, h, t0:t0 + TG], avh,
                                                 bcs)
                        return emit_epi
                    deferred.append(mk())

            for G in range(NG):
                emit_attn(G)
                if G == 0 and rope3_math[0] is not None:
                    rope3_math[0]()

            # ================= phase C: o_proj =================
            # one deferred epilogue (group 3 head 3) remains; flush it after
            # the first o_proj psum block (tag A0) so its reciprocal gets
            # cover, before the A1 block that reuses head 3's av bank
            for ti in range(NK):
                for eg in range(E // TG):
                    o_ps = ps.tile([128, TG], F32, name="o_ps",
                                   tag=["A0", "A1", "A4", "A5"][(ti * 8 + eg) % 4])
                    for h in range(HL):
                        nc.tensor.matmul(
                            o_ps, ao[:, h, 128 * ti:128 * (ti + 1)],
                            wo_sb[:, h, TG * eg:TG * (eg + 1)],
                            start=(h == 0), stop=(h == HL - 1))
                    while deferred:
                        deferred.pop(0)()
                    ob = obp.tile([128, TG], BF16, tag="ob", bufs=6)
                    if eg % 2 == 0:
                        nc.scalar.copy(out=ob, in_=o_ps)
                    else:
                        nc.vector.tensor_copy(ob, o_ps)
                    dq = [nc.gpsimd, nc.sync, nc.scalar][eg % 3]
                    dq.dma_start(out=outp_d[ti, eg], in_=ob)

    nc.compile()
    return nc


_CAUSAL_MASK_TILES = None


def _causal_mask_tiles():
    global _CAUSAL_MASK_TILES
    if _CAUSAL_MASK_TILES is None:
        kp = np.arange(128)[:, None]
        qc = np.arange(128)[None, :]
        _CAUSAL_MASK_TILES = np.where(qc >= kp, 0.0, NEG).astype(np.float32)
    return _CAUSAL_MASK_TILES


def _rope_tables(position_ids):
    pos = np.asarray(position_ids[0]).astype(np.float32)          # [S]
    inv_freq = (1.0 / (10000.0 ** (np.arange(0, D, 2, dtype=np.float32) / D)))
    freqs = pos[:, None] * inv_freq[None, :]                      # [S, 64]
    emb = np.concatenate([freqs, freqs], axis=1)                  # [S, 128]
    cosT = np.cos(emb).T.astype(np.float32)                       # [128, S]
    sinT = np.sin(emb).T.astype(np.float32)
    sinflipT = np.concatenate([-sinT[:64], sinT[64:]], axis=0).astype(np.float32)
    return np.ascontiguousarray(cosT), np.ascontiguousarray(sinflipT)


def kernel(hidden_states, position_ids, attention_mask, Wq, Wk, Wv, Wo):
    hidden_states = np.asarray(hidden_states)
    B = hidden_states.shape[0]
    assert hidden_states.shape == (B, S, E), hidden_states.shape
    assert B == 1

    mask = np.asarray(attention_mask, dtype=np.float32)[0, 0]
    if not mask.any():
        mode = "full"
    elif np.array_equal(mask, np.triu(np.full((S, S), NEG, dtype=np.float32), 1)):
        mode = "causal"
    else:
        mode = "general"

    if mode not in _PROGRAMS:
        _PROGRAMS[mode] = _build_program(mode)
    nc = _PROGRAMS[mode]

    hs = np.asarray(hidden_states[0], dtype=np.float32)
    # [E, S] -> group-major [NG, E, TG], bf16
    hsT = np.ascontiguousarray(
        hs.T.reshape(E, NG, TG).transpose(1, 0, 2)).astype(NPBF)
    cosT, sinflipT = _rope_tables(np.asarray(position_ids))
    # fold the 1/sqrt(D) score scaling into Wq so q and k share rope tables
    Wq = np.asarray(Wq, dtype=np.float32) * np.float32(1.0 / np.sqrt(D))
    Wk = np.asarray(Wk, dtype=np.float32)
    Wv = np.asarray(Wv, dtype=np.float32)
    Wo = np.asarray(Wo, dtype=np.float32)

    in_maps = []
    for c in range(NCORES):
        wqkv = np.concatenate([
            Wq[512 * c:512 * (c + 1), :].T,
            Wk[128 * c:128 * (c + 1), :].T,
            Wv[128 * c:128 * (c + 1), :].T,
        ], axis=1)
        m = {
            "hsT": hsT,
            "wqkvT": np.ascontiguousarray(wqkv).astype(NPBF),
            "woT": np.ascontiguousarray(Wo[:, 512 * c:512 * (c + 1)].T).astype(NPBF),
            "cosT": cosT, "sinT": sinflipT,
        }
        if mode == "causal":
            m["cmask"] = _causal_mask_tiles()
        elif mode == "general":
            m["maskT"] = np.ascontiguousarray(mask.T)
        in_maps.append(m)

    res = run_bass_kernel_spmd(nc, in_maps, core_ids=list(range(NCORES)),
                               trace=TRACE[0])
    LAST_EXEC_NS[0] = res.exec_time_ns
    LAST_RES[0] = res

    acc = np.zeros((NK, E // TG, 128, TG), dtype=np.float32)
    for c in range(NCORES):
        acc += res.results[c]["outp"].astype(np.float32)
    out = acc.transpose(0, 2, 1, 3).reshape(S, E)
    return out[None, :, :]


# revision 21
# speedup vs baseline: 1.0160x; 1.0160x over previous
"""Llama GQA attention layer (B=1, S=2048, E=4096, H=32, HKV=8, D=128) on 8
Trainium2 NeuronCores.

Sharding: tensor-parallel over heads. Core c owns Q heads 4c..4c+3 and KV head
c (KV groups stay intact), plus the matching Wo input-dim slice. Each core
computes a full [S, E] partial of the o_proj output in bf16; the host sums the
8 partials (the "all-reduce after o_proj").

All matmuls run in bf16 (1 cyc/row on the PE at 512-wide moving dim, with
automatic fast-weight-load; fp32r streams at ~1.3 cyc/row and pays 225ns
weight loads). PSUM accumulation stays fp32. l2 error budget is 2e-2; bf16
rounding of inputs/weights/probabilities lands well under 1e-2.

Per-core dataflow:
  phase A (PE-dense): per token group g (512 tokens):
    qT/kT/vT = W @ hs.T    6 psum chains x 32 E-chunks, [feat, tok] layout.
    Wq|Wk|Wv are host-packed into one [E, 768] tensor so each E-chunk is a
    single contiguous DMA and the first matmul fires ~2us after launch.
    RoPE off-PE: psum -> sbuf copy (scalar), half-swap via SBUF->SBUF DMA,
    cos/sin muls (DVE) -> qro/krope bf16. v: psum -> bf16 sbuf (scalar),
    PE-transposed to vnat [tok, d] between later groups' QKV streams.
  phase B attention, per query group G (causal: key tiles ki <= 4G+3, with
  moving-dim trimming + triangular mask add on diagonal tiles):
    scoresT[k, q] = krope_tile^T @ qro  (PSUM), exp on ScalarE -> bf16 expT
    avT[d, q]  accumulated over ki on PE (vnat stationary)
    den[h]     accumulated over ki on PE (ones[128,1] stationary, ~free
               weight load); all 4 heads share one psum bank at partition
               offsets 0/32/64/96.
    epilogue per head: DVE reciprocal on the [1, 512] den row only (full-tile
    reciprocal costs ~12 cyc/elem), K=1 ones matmul broadcasts it to 128
    partitions, DVE mul -> aoT bf16 (kept in SBUF, no DRAM spill).
  phase C o_proj: out[t, e] = sum_h aoT[:, h-tile]^T @ woT[h], 3 psum
    banks rotating, drains alternate scalar/vector, bf16 partials to DRAM.
"""

import sys
import types

if "/opt/trn_rl_repo" not in sys.path:
    sys.path.insert(0, "/opt/trn_rl_repo")

import numpy as np
import ml_dtypes

import concourse.bass as bass
import concourse.tile as tile
from concourse import bacc, mybir
from concourse.bass_utils import run_bass_kernel_spmd
from concourse.masks import make_identity

F32 = mybir.dt.float32
BF16 = mybir.dt.bfloat16
EXP = mybir.ActivationFunctionType.Exp
NPBF = ml_dtypes.bfloat16

S = 2048
E = 4096
H = 32
HKV = 8
D = 128
NCORES = 8
HL = H // NCORES          # 4 local q heads per core
TG = 512                  # token group (moving-dim tile)
NG = S // TG              # 4 token groups
NE = E // 128             # 32 contraction chunks
NK = S // 128             # 16 key tiles
FQKV = HL * D + 2 * D     # 768 packed output features per core
NEG = -1e9

TRACE = [False]
LAST_EXEC_NS = [None]
LAST_RES = [None]

_PROGRAMS = {}


def _install_ntff_hook():
    if "antenv.axon_hooks" in sys.modules:
        return
    mod = types.ModuleType("antenv.axon_hooks")
    hook = [None]
    mod.set_axon_ntff_profile_hook = lambda h: hook.__setitem__(0, h)
    mod.get_axon_ntff_profile_hook = lambda: hook[0]
    sys.modules["antenv.axon_hooks"] = mod
    try:
        from trn_agent_boot.trn_boot import _ntff_profile_via_ctypes

        mod.set_axon_ntff_profile_hook(
            _ntff_profile_via_ctypes("/opt/axon/libaxon_pjrt.so"))
    except Exception:
        pass


def set_trace(on=True):
    if on:
        _install_ntff_hook()
    TRACE[0] = on


def _build_program(mode):
    """mode: 'causal' (skip above-diagonal key tiles, trim + triangular mask
    on diagonal tiles), 'full' (no mask), 'general' (additive mask streamed
    from DRAM)."""
    nc = bacc.Bacc(trn_type="TRN2", target_bir_lowering=False, debug=False)

    # group-major hsT: [g, E, TG] so each [128, TG] chunk is contiguous
    hsT_d = nc.dram_tensor("hsT", [NG, E, TG], BF16, kind="ExternalInput").ap()
    # packed [Wq | Wk | Wv] transposed: rows are E, cols 768
    wqkv_d = nc.dram_tensor("wqkvT", [E, FQKV], BF16, kind="ExternalInput").ap()
    woT_d = nc.dram_tensor("woT", [HL * D, E], BF16, kind="ExternalInput").ap()
    cos_d = nc.dram_tensor("cosT", [D, S], F32, kind="ExternalInput").ap()
    sin_d = nc.dram_tensor("sinT", [D, S], F32, kind="ExternalInput").ap()
    if mode == "causal":
        cmask_d = nc.dram_tensor("cmask", [128, 128], F32,
                                 kind="ExternalInput").ap()
    elif mode == "general":
        maskT_d = nc.dram_tensor("maskT", [S, S], F32, kind="ExternalInput").ap()
    # tile-major output: [ti, eg, 128, TG] so each store is contiguous
    outp_d = nc.dram_tensor("outp", [NK, E // TG, 128, TG], BF16,
                            kind="ExternalOutput").ap()

    with tile.TileContext(nc) as tc:
        with tc.tile_pool(name="const", bufs=1) as cpool, \
             tc.tile_pool(name="persist", bufs=1) as pp, \
             tc.tile_pool(name="wqkv", bufs=1) as wp, \
             tc.tile_pool(name="cs", bufs=1) as csp, \
             tc.tile_pool(name="hst", bufs=8) as hp, \
             tc.tile_pool(name="rope", bufs=1) as rp, \
             tc.tile_pool(name="attn", bufs=1) as ap_, \
             tc.tile_pool(name="outb", bufs=1) as obp, \
             tc.tile_pool(name="ps", bufs=1, space="PSUM") as ps:

            # ---- constants ----
            identf = cpool.tile([128, 128], F32)
            make_identity(nc, identf)
            ident = cpool.tile([128, 128], BF16)
            nc.vector.tensor_copy(ident, identf)
            onesf = cpool.tile([128, 32], F32)
            nc.vector.memset(onesf, 1.0)
            ones_col = cpool.tile([128, 32], BF16)
            nc.vector.tensor_copy(ones_col, onesf)
            # ones rows at partition bases 0 and 32 (matmul operands must
            # share a 32-aligned base partition with the den rows they read)
            onesrf = cpool.tile([64, 128], F32)
            nc.vector.memset(onesrf, 1.0)
            ones_rows = cpool.tile([64, 128], BF16)
            nc.vector.tensor_copy(ones_rows, onesrf)
            if mode == "causal":
                cmask = cpool.tile([128, 128], F32)

            # ---- persistent activations ----
            krope = pp.tile([128, S], BF16)               # [d, tok]
            vnat = pp.tile([128, NK, 128], BF16)          # [tok%128, ktile, d]
            ao = pp.tile([128, HL, S], BF16)              # [d, head, tok]
            qro = pp.tile([128, NG, HL, TG], BF16)        # [d, g, head, tok]

            # ---- weights: chunk loads interleaved with group-0 hsT so the
            # first QKV matmul fires almost immediately ----
            w_sb = wp.tile([128, NE, FQKV], BF16)
            wo_sb = wp.tile([128, HL, E], BF16)
            cos_sb = csp.tile([128, S], F32)
            sin_sb = csp.tile([128, S], F32)

            # weight chunks batched x4 on the scalar queue, group-0 hsT on the
            # sync queue: both pipelines issue in parallel so the first
            # matmul fires as early as possible (each dma_start costs ~0.6us
            # of descriptor generation on its issuing queue)
            wqkv_r = wqkv_d.rearrange("(ne p) f -> p ne f", p=128)
            hst0 = []
            for e in range(NE):
                if e % 4 == 0:
                    nc.scalar.dma_start(out=w_sb[:, e:e + 4, :],
                                        in_=wqkv_r[:, e:e + 4, :])
                hst = hp.tile([128, TG], BF16, tag="hst")
                nc.sync.dma_start(
                    out=hst, in_=hsT_d[0, 128 * e:128 * (e + 1), :])
                hst0.append(hst)
            # cos/sin (2MB) + cmask load after the weights; not needed
            # until rope(0)/attention
            nc.scalar.dma_start(out=cos_sb, in_=cos_d)
            nc.scalar.dma_start(out=sin_sb, in_=sin_d)
            if mode == "causal":
                nc.scalar.dma_start(out=cmask, in_=cmask_d)

            # ================= phase A: QKV projection + RoPE =================
            def emit_qkv(g):
                q_ps = [ps.tile([128, TG], F32, tag=f"A{f}", name=f"q_ps{f}")
                        for f in range(HL)]
                k_ps = ps.tile([128, TG], F32, tag="A4", name="k_ps")
                v_ps = ps.tile([128, TG], F32, tag="A5", name="v_ps")
                for e in range(NE):
                    if g == 0:
                        hst = hst0[e]
                    else:
                        hst = hp.tile([128, TG], BF16, tag="hst")
                        nc.sync.dma_start(
                            out=hst, in_=hsT_d[g, 128 * e:128 * (e + 1), :])
                    st, sp = (e == 0), (e == NE - 1)
                    for f in range(HL):
                        nc.tensor.matmul(
                            q_ps[f], w_sb[:, e, 128 * f:128 * (f + 1)],
                            hst, start=st, stop=sp)
                    nc.tensor.matmul(k_ps, w_sb[:, e, 512:640], hst,
                                     start=st, stop=sp)
                    nc.tensor.matmul(v_ps, w_sb[:, e, 640:768], hst,
                                     start=st, stop=sp)
                return q_ps, k_ps, v_ps

            def emit_rope(g, q_ps, k_ps, v_ps):
                t0 = g * TG
                cs = cos_sb[:, t0:t0 + TG]
                sn = sin_sb[:, t0:t0 + TG]
                pairs = [(q_ps[f], qro[:, g, f, :]) for f in range(HL)]
                kpair = (k_ps, krope[:, t0:t0 + TG])
                # last group: drain k first so attention's first score matmul
                # (s-bank reuse of the k psum tag) unblocks earliest
                pairs = [kpair] + pairs if g == NG - 1 else pairs + [kpair]
                # drain ALL six psum banks first, copies split across scalar
                # and vector so the next group's QKV matmuls unblock in ~1us;
                # the rope math below then reads only SBUF
                vs = rp.tile([128, TG], BF16, tag="vs", bufs=2)
                if g == NG - 1:
                    nc.scalar.copy(out=vs, in_=v_ps)
                xss = []
                for i, (x_ps, _) in enumerate(pairs):
                    xs = rp.tile([128, TG], F32, tag="xs", bufs=6)
                    # last group: keep the scalar queue clear for attention's
                    # exps -- only k drains there, the q heads go to vector
                    if (i == 0) if g == NG - 1 else (i % 2 == 0):
                        nc.scalar.copy(out=xs, in_=x_ps)
                    else:
                        nc.vector.tensor_copy(xs, x_ps)
                    xss.append(xs)
                if g != NG - 1:
                    nc.scalar.copy(out=vs, in_=v_ps)

                def math():
                    for xs, (_, out_ap) in zip(xss, pairs):
                        swp = rp.tile([128, TG], F32, tag="swp", bufs=3)
                        nc.gpsimd.dma_start(out=swp[0:64, :], in_=xs[64:128, :])
                        nc.gpsimd.dma_start(out=swp[64:128, :], in_=xs[0:64, :])
                        p1 = rp.tile([128, TG], F32, tag="p1", bufs=2)
                        nc.vector.tensor_mul(p1, xs, cs)
                        nc.vector.tensor_mul(swp, swp, sn)
                        nc.vector.tensor_add(out_ap, p1, swp)
                # group 3's rope outputs are only read by attention(3): defer
                # its math emission behind attention(1) so groups 0/1 (short
                # head streams, every tile diagonal-masked) don't queue their
                # cmask adds and reciprocals behind it on the vector engine
                if g == NG - 1:
                    return vs, math
                math()
                return vs, None

            def emit_vtr(g, vs):
                for j in range(4):
                    tr = ps.tile([128, 128], BF16, tag="A6", name="tr_ps")
                    nc.tensor.transpose(tr, vs[:, 128 * j:128 * (j + 1)], ident)
                    nc.vector.tensor_copy(vnat[:, 4 * g + j, :], tr)

            vs_pend = []
            rope3_math = [None]
            for g in range(NG):
                qkv = emit_qkv(g)
                if vs_pend:
                    emit_vtr(*vs_pend.pop())
                vs, m = emit_rope(g, *qkv)
                rope3_math[0] = m
                vs_pend.append((g, vs))
            emit_vtr(*vs_pend.pop())

            # wo loads issue from the (otherwise idle) gpsimd queue during
            # attention
            woT_r = woT_d.rearrange("(h p) e -> p h e", p=128)
            for eg in range(E // TG):
                nc.gpsimd.dma_start(
                    out=wo_sb[:, :, TG * eg:TG * (eg + 1)],
                    in_=woT_r[:, :, TG * eg:TG * (eg + 1)])

            # ================= phase B: attention =================
            # Head-major: each head's full key sweep completes before the next
            # head begins, so the per-head softmax epilogue (3.3us flat DVE
            # reciprocal) overlaps the NEXT head's matmul stream instead of
            # four reciprocals serializing at the group boundary. Banks:
            # av alternates A0/A1, scores alternate A4/A5, each head owns a
            # private den bank (A2/A3/A6/A7) whose tag is reused for its own
            # broadcast matmul. Epilogues defer by at most one head: the
            # flush invariant keeps every bank's previous reader emitted
            # before its next writer (PE-queue deadlock freedom).
            deferred = []

            def emit_attn(G):
                nk = 4 * G + 4 if mode == "causal" else NK
                t0 = G * TG
                for h in range(HL):
                    while len(deferred) > 1:
                        deferred.pop(0)()
                    av = ps.tile([128, TG], F32, tag=["A0", "A1"][h % 2],
                                 name=f"av{h}")
                    den = ps.tile([128, TG], F32, name="den",
                                  tag=["A2", "A3", "A6", "A7"][h])
                    pend = []

                    def drain(item, av=av, den=den, nk=nk):
                        ki, c0, ex = item
                        nc.tensor.matmul(av[:, c0:], vnat[:, ki, :],
                                         ex[:, c0:], start=(ki == 0),
                                         stop=(ki == nk - 1),
                                         skip_group_check=True)
                        nc.tensor.matmul(den[0:32, c0:], ones_col,
                                         ex[:, c0:],
                                         start=(ki == 0), stop=(ki == nk - 1),
                                         skip_group_check=True)

                    for ki in range(nk):
                        c0 = max(0, 128 * ki - TG * G) if mode == "causal" else 0
                        s = ps.tile([128, TG], F32, name="s_ps",
                                    tag=["A4", "A5"][ki % 2])
                        nc.tensor.matmul(s[:, c0:],
                                         krope[:, 128 * ki:128 * (ki + 1)],
                                         qro[:, G, h, c0:],
                                         start=True, stop=True)
                        if mode == "causal" and ki >= 4 * G:
                            nc.vector.tensor_add(s[:, c0:c0 + 128],
                                                 s[:, c0:c0 + 128], cmask)
                        elif mode == "general":
                            mt = ap_.tile([128, TG], F32, tag="mt", bufs=4)
                            nc.sync.dma_start(
                                out=mt, in_=maskT_d[128 * ki:128 * (ki + 1),
                                                    TG * G:TG * (G + 1)])
                            nc.vector.tensor_add(s, s, mt)
                        ex = ap_.tile([128, TG], BF16, tag="ex", bufs=8)
                        nc.scalar.activation(out=ex[:, c0:], in_=s[:, c0:],
                                             func=EXP)
                        pend.append((ki, c0, ex))
                        # consume the previous head's epilogue only once its
                        # reciprocal has had a few microseconds of cover
                        if ki >= 6 and deferred:
                            deferred.pop(0)()
                        while len(pend) > 2:
                            drain(pend.pop(0))
                    while pend:
                        drain(pend.pop(0))
                    # reciprocal fires now, hidden under the next head's
                    # matmuls; the psum-side epilogue is deferred
                    rcb = ap_.tile([64, TG], BF16, tag="rcb", bufs=3)
                    with nc.allow_low_precision(reason="softmax recip"):
                        nc.vector.reciprocal(rcb[0:1, :], den[0:1, :])
                    bc = ps.tile([128, TG], F32, name="bc_ps",
                                 tag=["A2", "A3", "A6", "A7"][h])

                    def mk(h=h, rcb=rcb, bc=bc, avh=av, t0=t0):
                        def emit_epi():
                            nc.tensor.matmul(bc, ones_rows[0:1, :],
                                             rcb[0:1, :], start=True,
                                             stop=True)
                            bcs = ap_.tile([128, TG], BF16, tag="bcs", bufs=2)
                            nc.vector.tensor_copy(bcs, bc)
                            nc.vector.tensor_mul(ao[:, h, t0:t0 + TG], avh,
                                                 bcs)
                        return emit_epi
                    deferred.append(mk())

            for G in range(NG):
                emit_attn(G)
                if G == 1 and rope3_math[0] is not None:
                    rope3_math[0]()

            # ================= phase C: o_proj =================
            # one deferred epilogue (group 3 head 3) remains; flush it after
            # the first o_proj psum block (tag A0) so its reciprocal gets
            # cover, before the A1 block that reuses head 3's av bank
            for ti in range(NK):
                for eg in range(E // TG):
                    o_ps = ps.tile([128, TG], F32, name="o_ps",
                                   tag=["A0", "A1", "A4", "A5"][(ti * 8 + eg) % 4])
                    for h in range(HL):
                        nc.tensor.matmul(
                            o_ps, ao[:, h, 128 * ti:128 * (ti + 1)],
                            wo_sb[:, h, TG * eg:TG * (eg + 1)],
                            start=(h == 0), stop=(h == HL - 1))
                    while deferred:
                        deferred.pop(0)()
                    ob = obp.tile([128, TG], BF16, tag="ob", bufs=6)
                    if eg % 2 == 0:
                        nc.scalar.copy(out=ob, in_=o_ps)
                    else:
                        nc.vector.tensor_copy(ob, o_ps)
                    if eg % 2 == 0:
                        nc.gpsimd.dma_start(out=outp_d[ti, eg], in_=ob)
                    else:
                        nc.sync.dma_start(out=outp_d[ti, eg], in_=ob)

    nc.compile()
    return nc


_CAUSAL_MASK_TILES = None


def _causal_mask_tiles():
    global _CAUSAL_MASK_TILES
    if _CAUSAL_MASK_TILES is None:
        kp = np.arange(128)[:, None]
        qc = np.arange(128)[None, :]
        _CAUSAL_MASK_TILES = np.where(qc >= kp, 0.0, NEG).astype(np.float32)
    return _CAUSAL_MASK_TILES


def _rope_tables(position_ids):
    pos = np.asarray(position_ids[0]).astype(np.float32)          # [S]
    inv_freq = (1.0 / (10000.0 ** (np.arange(0, D, 2, dtype=np.float32) / D)))
    freqs = pos[:, None] * inv_freq[None, :]                      # [S, 64]
    emb = np.concatenate([freqs, freqs], axis=1)                  # [S, 128]
    cosT = np.cos(emb).T.astype(np.float32)                       # [128, S]
    sinT = np.sin(emb).T.astype(np.float32)
    sinflipT = np.concatenate([-sinT[:64], sinT[64:]], axis=0).astype(np.float32)
    return np.ascontiguousarray(cosT), np.ascontiguousarray(sinflipT)


def kernel(hidden_states, position_ids, attention_mask, Wq, Wk, Wv, Wo):
    hidden_states = np.asarray(hidden_states)
    B = hidden_states.shape[0]
    assert hidden_states.shape == (B, S, E), hidden_states.shape
    assert B == 1

    mask = np.asarray(attention_mask, dtype=np.float32)[0, 0]
    if not mask.any():
        mode = "full"
    elif np.array_equal(mask, np.triu(np.full((S, S), NEG, dtype=np.float32), 1)):
        mode = "causal"
    else:
        mode = "general"

    if mode not in _PROGRAMS:
        _PROGRAMS[mode] = _build_program(mode)
    nc = _PROGRAMS[mode]

    hs = np.asarray(hidden_states[0], dtype=np.float32)
    # [E, S] -> group-major [NG, E, TG], bf16
    hsT = np.ascontiguousarray(
        hs.T.reshape(E, NG, TG).transpose(1, 0, 2)).astype(NPBF)
    cosT, sinflipT = _rope_tables(np.asarray(position_ids))
    # fold the 1/sqrt(D) score scaling into Wq so q and k share rope tables
    Wq = np.asarray(Wq, dtype=np.float32) * np.float32(1.0 / np.sqrt(D))
    Wk = np.asarray(Wk, dtype=np.float32)
    Wv = np.asarray(Wv, dtype=np.float32)
    Wo = np.asarray(Wo, dtype=np.float32)

    in_maps = []
    for c in range(NCORES):
        wqkv = np.concatenate([
            Wq[512 * c:512 * (c + 1), :].T,
            Wk[128 * c:128 * (c + 1), :].T,
            Wv[128 * c:128 * (c + 1), :].T,
        ], axis=1)
        m = {
            "hsT": hsT,
            "wqkvT": np.ascontiguousarray(wqkv).astype(NPBF),
            "woT": np.ascontiguousarray(Wo[:, 512 * c:512 * (c + 1)].T).astype(NPBF),
            "cosT": cosT, "sinT": sinflipT,
        }
        if mode == "causal":
            m["cmask"] = _causal_mask_tiles()
        elif mode == "general":
            m["maskT"] = np.ascontiguousarray(mask.T)
        in_maps.append(m)

    res = run_bass_kernel_spmd(nc, in_maps, core_ids=list(range(NCORES)),
                               trace=TRACE[0])
    LAST_EXEC_NS[0] = res.exec_time_ns
    LAST_RES[0] = res

    acc = np.zeros((NK, E // TG, 128, TG), dtype=np.float32)
    for c in range(NCORES):
        acc += res.results[c]["outp"].astype(np.float32)
    out = acc.transpose(0, 2, 1, 3).reshape(S, E)
    return out[None, :, :]


# revision 22
# speedup vs baseline: 1.0229x; 1.0068x over previous
"""Llama GQA attention layer (B=1, S=2048, E=4096, H=32, HKV=8, D=128) on 8
Trainium2 NeuronCores.

Sharding: tensor-parallel over heads. Core c owns Q heads 4c..4c+3 and KV head
c (KV groups stay intact), plus the matching Wo input-dim slice. Each core
computes a full [S, E] partial of the o_proj output in bf16; the host sums the
8 partials (the "all-reduce after o_proj").

All matmuls run in bf16 (1 cyc/row on the PE at 512-wide moving dim, with
automatic fast-weight-load; fp32r streams at ~1.3 cyc/row and pays 225ns
weight loads). PSUM accumulation stays fp32. l2 error budget is 2e-2; bf16
rounding of inputs/weights/probabilities lands well under 1e-2.

Per-core dataflow:
  phase A (PE-dense): per token group g (512 tokens):
    qT/kT/vT = W @ hs.T    6 psum chains x 32 E-chunks, [feat, tok] layout.
    Wq|Wk|Wv are host-packed into one [E, 768] tensor so each E-chunk is a
    single contiguous DMA and the first matmul fires ~2us after launch.
    RoPE off-PE: psum -> sbuf copy (scalar), half-swap via SBUF->SBUF DMA,
    cos/sin muls (DVE) -> qro/krope bf16. v: psum -> bf16 sbuf (scalar),
    PE-transposed to vnat [tok, d] between later groups' QKV streams.
  phase B attention, per query group G (causal: key tiles ki <= 4G+3, with
  moving-dim trimming + triangular mask add on diagonal tiles):
    scoresT[k, q] = krope_tile^T @ qro  (PSUM), exp on ScalarE -> bf16 expT
    avT[d, q]  accumulated over ki on PE (vnat stationary)
    den[h]     accumulated over ki on PE (ones[128,1] stationary, ~free
               weight load); all 4 heads share one psum bank at partition
               offsets 0/32/64/96.
    epilogue per head: DVE reciprocal on the [1, 512] den row only (full-tile
    reciprocal costs ~12 cyc/elem), K=1 ones matmul broadcasts it to 128
    partitions, DVE mul -> aoT bf16 (kept in SBUF, no DRAM spill).
  phase C o_proj: out[t, e] = sum_h aoT[:, h-tile]^T @ woT[h], 3 psum
    banks rotating, drains alternate scalar/vector, bf16 partials to DRAM.
"""

import sys
import types

if "/opt/trn_rl_repo" not in sys.path:
    sys.path.insert(0, "/opt/trn_rl_repo")

import numpy as np
import ml_dtypes

import concourse.bass as bass
import concourse.tile as tile
from concourse import bacc, mybir
from concourse.bass_utils import run_bass_kernel_spmd
from concourse.masks import make_identity

F32 = mybir.dt.float32
BF16 = mybir.dt.bfloat16
EXP = mybir.ActivationFunctionType.Exp
NPBF = ml_dtypes.bfloat16

S = 2048
E = 4096
H = 32
HKV = 8
D = 128
NCORES = 8
HL = H // NCORES          # 4 local q heads per core
TG = 512                  # token group (moving-dim tile)
NG = S // TG              # 4 token groups
NE = E // 128             # 32 contraction chunks
NK = S // 128             # 16 key tiles
FQKV = HL * D + 2 * D     # 768 packed output features per core
NEG = -1e9

TRACE = [False]
LAST_EXEC_NS = [None]
LAST_RES = [None]

_PROGRAMS = {}


def _install_ntff_hook():
    if "antenv.axon_hooks" in sys.modules:
        return
    mod = types.ModuleType("antenv.axon_hooks")
    hook = [None]
    mod.set_axon_ntff_profile_hook = lambda h: hook.__setitem__(0, h)
    mod.get_axon_ntff_profile_hook = lambda: hook[0]
    sys.modules["antenv.axon_hooks"] = mod
    try:
        from trn_agent_boot.trn_boot import _ntff_profile_via_ctypes

        mod.set_axon_ntff_profile_hook(
            _ntff_profile_via_ctypes("/opt/axon/libaxon_pjrt.so"))
    except Exception:
        pass


def set_trace(on=True):
    if on:
        _install_ntff_hook()
    TRACE[0] = on


def _build_program(mode):
    """mode: 'causal' (skip above-diagonal key tiles, trim + triangular mask
    on diagonal tiles), 'full' (no mask), 'general' (additive mask streamed
    from DRAM)."""
    nc = bacc.Bacc(trn_type="TRN2", target_bir_lowering=False, debug=False)

    # group-major hsT: [g, E, TG] so each [128, TG] chunk is contiguous
    hsT_d = nc.dram_tensor("hsT", [NG, E, TG], BF16, kind="ExternalInput").ap()
    # packed [Wq | Wk | Wv] transposed: rows are E, cols 768
    wqkv_d = nc.dram_tensor("wqkvT", [E, FQKV], BF16, kind="ExternalInput").ap()
    woT_d = nc.dram_tensor("woT", [HL * D, E], BF16, kind="ExternalInput").ap()
    cos_d = nc.dram_tensor("cosT", [D, S], F32, kind="ExternalInput").ap()
    sin_d = nc.dram_tensor("sinT", [D, S], F32, kind="ExternalInput").ap()
    if mode == "causal":
        cmask_d = nc.dram_tensor("cmask", [128, 128], F32,
                                 kind="ExternalInput").ap()
    elif mode == "general":
        maskT_d = nc.dram_tensor("maskT", [S, S], F32, kind="ExternalInput").ap()
    # tile-major output: [ti, eg, 128, TG] so each store is contiguous
    outp_d = nc.dram_tensor("outp", [NK, E // TG, 128, TG], BF16,
                            kind="ExternalOutput").ap()

    with tile.TileContext(nc) as tc:
        with tc.tile_pool(name="const", bufs=1) as cpool, \
             tc.tile_pool(name="persist", bufs=1) as pp, \
             tc.tile_pool(name="wqkv", bufs=1) as wp, \
             tc.tile_pool(name="cs", bufs=1) as csp, \
             tc.tile_pool(name="hst", bufs=12) as hp, \
             tc.tile_pool(name="rope", bufs=1) as rp, \
             tc.tile_pool(name="attn", bufs=1) as ap_, \
             tc.tile_pool(name="outb", bufs=1) as obp, \
             tc.tile_pool(name="ps", bufs=1, space="PSUM") as ps:

            # ---- constants ----
            identf = cpool.tile([128, 128], F32)
            make_identity(nc, identf)
            ident = cpool.tile([128, 128], BF16)
            nc.vector.tensor_copy(ident, identf)
            onesf = cpool.tile([128, 32], F32)
            nc.vector.memset(onesf, 1.0)
            ones_col = cpool.tile([128, 32], BF16)
            nc.vector.tensor_copy(ones_col, onesf)
            # ones rows at partition bases 0 and 32 (matmul operands must
            # share a 32-aligned base partition with the den rows they read)
            onesrf = cpool.tile([64, 128], F32)
            nc.vector.memset(onesrf, 1.0)
            ones_rows = cpool.tile([64, 128], BF16)
            nc.vector.tensor_copy(ones_rows, onesrf)
            if mode == "causal":
                cmask = cpool.tile([128, 128], F32)

            # ---- persistent activations ----
            krope = pp.tile([128, S], BF16)               # [d, tok]
            vnat = pp.tile([128, NK, 128], BF16)          # [tok%128, ktile, d]
            ao = pp.tile([128, HL, S], BF16)              # [d, head, tok]
            qro = pp.tile([128, NG, HL, TG], BF16)        # [d, g, head, tok]

            # ---- weights: chunk loads interleaved with group-0 hsT so the
            # first QKV matmul fires almost immediately ----
            w_sb = wp.tile([128, NE, FQKV], BF16)
            wo_sb = wp.tile([128, HL, E], BF16)
            cos_sb = csp.tile([128, S], F32)
            sin_sb = csp.tile([128, S], F32)

            # weight chunks batched x4 on the scalar queue, group-0 hsT on the
            # sync queue: both pipelines issue in parallel so the first
            # matmul fires as early as possible (each dma_start costs ~0.6us
            # of descriptor generation on its issuing queue)
            wqkv_r = wqkv_d.rearrange("(ne p) f -> p ne f", p=128)
            hst0 = []
            for e in range(NE):
                if e % 4 == 0:
                    nc.scalar.dma_start(out=w_sb[:, e:e + 4, :],
                                        in_=wqkv_r[:, e:e + 4, :])
                hst = hp.tile([128, TG], BF16, tag="hst")
                nc.sync.dma_start(
                    out=hst, in_=hsT_d[0, 128 * e:128 * (e + 1), :])
                hst0.append(hst)
            # cos/sin (2MB) + cmask load after the weights; not needed
            # until rope(0)/attention
            nc.scalar.dma_start(out=cos_sb, in_=cos_d)
            nc.scalar.dma_start(out=sin_sb, in_=sin_d)
            if mode == "causal":
                nc.scalar.dma_start(out=cmask, in_=cmask_d)

            # ================= phase A: QKV projection + RoPE =================
            def emit_qkv(g):
                q_ps = [ps.tile([128, TG], F32, tag=f"A{f}", name=f"q_ps{f}")
                        for f in range(HL)]
                k_ps = ps.tile([128, TG], F32, tag="A4", name="k_ps")
                v_ps = ps.tile([128, TG], F32, tag="A5", name="v_ps")
                for e in range(NE):
                    if g == 0:
                        hst = hst0[e]
                    else:
                        hst = hp.tile([128, TG], BF16, tag="hst")
                        nc.sync.dma_start(
                            out=hst, in_=hsT_d[g, 128 * e:128 * (e + 1), :])
                    st, sp = (e == 0), (e == NE - 1)
                    for f in range(HL):
                        nc.tensor.matmul(
                            q_ps[f], w_sb[:, e, 128 * f:128 * (f + 1)],
                            hst, start=st, stop=sp)
                    nc.tensor.matmul(k_ps, w_sb[:, e, 512:640], hst,
                                     start=st, stop=sp)
                    nc.tensor.matmul(v_ps, w_sb[:, e, 640:768], hst,
                                     start=st, stop=sp)
                return q_ps, k_ps, v_ps

            def emit_rope(g, q_ps, k_ps, v_ps):
                t0 = g * TG
                cs = cos_sb[:, t0:t0 + TG]
                sn = sin_sb[:, t0:t0 + TG]
                pairs = [(q_ps[f], qro[:, g, f, :]) for f in range(HL)]
                kpair = (k_ps, krope[:, t0:t0 + TG])
                # last group: drain k first so attention's first score matmul
                # (s-bank reuse of the k psum tag) unblocks earliest
                pairs = [kpair] + pairs if g == NG - 1 else pairs + [kpair]
                # drain ALL six psum banks first, copies split across scalar
                # and vector so the next group's QKV matmuls unblock in ~1us;
                # the rope math below then reads only SBUF
                vs = rp.tile([128, TG], BF16, tag="vs", bufs=2)
                if g == NG - 1:
                    nc.scalar.copy(out=vs, in_=v_ps)
                xss = []
                for i, (x_ps, _) in enumerate(pairs):
                    xs = rp.tile([128, TG], F32, tag="xs", bufs=6)
                    # last group: keep the scalar queue clear for attention's
                    # exps -- only k drains there, the q heads go to vector
                    if (i == 0) if g == NG - 1 else (i % 2 == 0):
                        nc.scalar.copy(out=xs, in_=x_ps)
                    else:
                        nc.vector.tensor_copy(xs, x_ps)
                    xss.append(xs)
                if g != NG - 1:
                    nc.scalar.copy(out=vs, in_=v_ps)

                def math():
                    for xs, (_, out_ap) in zip(xss, pairs):
                        swp = rp.tile([128, TG], F32, tag="swp", bufs=3)
                        nc.gpsimd.dma_start(out=swp[0:64, :], in_=xs[64:128, :])
                        nc.gpsimd.dma_start(out=swp[64:128, :], in_=xs[0:64, :])
                        p1 = rp.tile([128, TG], F32, tag="p1", bufs=2)
                        nc.vector.tensor_mul(p1, xs, cs)
                        nc.vector.tensor_mul(swp, swp, sn)
                        nc.vector.tensor_add(out_ap, p1, swp)
                # group 3's rope outputs are only read by attention(3): defer
                # its math emission behind attention(1) so groups 0/1 (short
                # head streams, every tile diagonal-masked) don't queue their
                # cmask adds and reciprocals behind it on the vector engine
                if g == NG - 1:
                    return vs, math
                math()
                return vs, None

            def emit_vtr(g, vs):
                for j in range(4):
                    tr = ps.tile([128, 128], BF16, tag="A6", name="tr_ps")
                    nc.tensor.transpose(tr, vs[:, 128 * j:128 * (j + 1)], ident)
                    nc.vector.tensor_copy(vnat[:, 4 * g + j, :], tr)

            vs_pend = []
            rope3_math = [None]
            for g in range(NG):
                qkv = emit_qkv(g)
                if vs_pend:
                    emit_vtr(*vs_pend.pop())
                vs, m = emit_rope(g, *qkv)
                rope3_math[0] = m
                vs_pend.append((g, vs))
            emit_vtr(*vs_pend.pop())

            # wo loads issue from the (otherwise idle) gpsimd queue during
            # attention
            woT_r = woT_d.rearrange("(h p) e -> p h e", p=128)
            for eg in range(E // TG):
                nc.gpsimd.dma_start(
                    out=wo_sb[:, :, TG * eg:TG * (eg + 1)],
                    in_=woT_r[:, :, TG * eg:TG * (eg + 1)])

            # ================= phase B: attention =================
            # Head-major: each head's full key sweep completes before the next
            # head begins, so the per-head softmax epilogue (3.3us flat DVE
            # reciprocal) overlaps the NEXT head's matmul stream instead of
            # four reciprocals serializing at the group boundary. Banks:
            # av alternates A0/A1, scores alternate A4/A5, each head owns a
            # private den bank (A2/A3/A6/A7) whose tag is reused for its own
            # broadcast matmul. Epilogues defer by at most one head: the
            # flush invariant keeps every bank's previous reader emitted
            # before its next writer (PE-queue deadlock freedom).
            deferred = []

            def emit_attn(G):
                nk = 4 * G + 4 if mode == "causal" else NK
                t0 = G * TG
                for h in range(HL):
                    av = ps.tile([128, TG], F32, tag=["A0", "A1"][h % 2],
                                 name=f"av{h}")
                    den = ps.tile([128, TG], F32, name="den",
                                  tag=["A2", "A3", "A6", "A7"][h])
                    pend = []

                    def drain(item, av=av, den=den, nk=nk):
                        ki, c0, ex = item
                        nc.tensor.matmul(av[:, c0:], vnat[:, ki, :],
                                         ex[:, c0:], start=(ki == 0),
                                         stop=(ki == nk - 1),
                                         skip_group_check=True)
                        nc.tensor.matmul(den[0:32, c0:], ones_col,
                                         ex[:, c0:],
                                         start=(ki == 0), stop=(ki == nk - 1),
                                         skip_group_check=True)

                    for ki in range(nk):
                        c0 = max(0, 128 * ki - TG * G) if mode == "causal" else 0
                        s = ps.tile([128, TG], F32, name="s_ps",
                                    tag=["A4", "A5"][ki % 2])
                        nc.tensor.matmul(s[:, c0:],
                                         krope[:, 128 * ki:128 * (ki + 1)],
                                         qro[:, G, h, c0:],
                                         start=True, stop=True)
                        if mode == "causal" and ki >= 4 * G:
                            nc.vector.tensor_add(s[:, c0:c0 + 128],
                                                 s[:, c0:c0 + 128], cmask)
                        elif mode == "general":
                            mt = ap_.tile([128, TG], F32, tag="mt", bufs=4)
                            nc.sync.dma_start(
                                out=mt, in_=maskT_d[128 * ki:128 * (ki + 1),
                                                    TG * G:TG * (G + 1)])
                            nc.vector.tensor_add(s, s, mt)
                        ex = ap_.tile([128, TG], BF16, tag="ex", bufs=10)
                        nc.scalar.activation(out=ex[:, c0:], in_=s[:, c0:],
                                             func=EXP)
                        pend.append((ki, c0, ex))
                        # consume the previous heads' epilogues as late as
                        # the bank-reuse deadlock rule allows (before this
                        # head's first av/den drain at ki==3), giving their
                        # reciprocals maximum cover
                        if ki == 2:
                            while len(deferred) > 1:
                                deferred.pop(0)()
                        elif ki >= 6 and deferred:
                            deferred.pop(0)()
                        while len(pend) > 2:
                            drain(pend.pop(0))
                    while pend:
                        drain(pend.pop(0))
                    # reciprocal fires now, hidden under the next head's
                    # matmuls; the psum-side epilogue is deferred
                    rcb = ap_.tile([64, TG], BF16, tag="rcb", bufs=3)
                    with nc.allow_low_precision(reason="softmax recip"):
                        nc.vector.reciprocal(rcb[0:1, :], den[0:1, :])
                    bc = ps.tile([128, TG], F32, name="bc_ps",
                                 tag=["A2", "A3", "A6", "A7"][h])

                    def mk(h=h, rcb=rcb, bc=bc, avh=av, t0=t0):
                        def emit_epi():
                            nc.tensor.matmul(bc, ones_rows[0:1, :],
                                             rcb[0:1, :], start=True,
                                             stop=True)
                            bcs = ap_.tile([128, TG], BF16, tag="bcs", bufs=2)
                            nc.vector.tensor_copy(bcs, bc)
                            nc.vector.tensor_mul(ao[:, h, t0:t0 + TG], avh,
                                                 bcs)
                        return emit_epi
                    deferred.append(mk())

            for G in range(NG):
                emit_attn(G)
                if G == 1 and rope3_math[0] is not None:
                    rope3_math[0]()

            # ================= phase C: o_proj =================
            # one deferred epilogue (group 3 head 3) remains; flush it after
            # the first o_proj psum block (tag A0) so its reciprocal gets
            # cover, before the A1 block that reuses head 3's av bank
            for ti in range(NK):
                for eg in range(E // TG):
                    o_ps = ps.tile([128, TG], F32, name="o_ps",
                                   tag=["A0", "A4", "A1", "A5"][(ti * 8 + eg) % 4])
                    for h in range(HL):
                        nc.tensor.matmul(
                            o_ps, ao[:, h, 128 * ti:128 * (ti + 1)],
                            wo_sb[:, h, TG * eg:TG * (eg + 1)],
                            start=(h == 0), stop=(h == HL - 1))
                    while deferred:
                        deferred.pop(0)()
                    ob = obp.tile([128, TG], BF16, tag="ob", bufs=6)
                    if eg % 2 == 0:
                        nc.scalar.copy(out=ob, in_=o_ps)
                    else:
                        nc.vector.tensor_copy(ob, o_ps)
                    if eg % 2 == 0:
                        nc.gpsimd.dma_start(out=outp_d[ti, eg], in_=ob)
                    else:
                        nc.sync.dma_start(out=outp_d[ti, eg], in_=ob)

    nc.compile()
    return nc


_CAUSAL_MASK_TILES = None


def _causal_mask_tiles():
    global _CAUSAL_MASK_TILES
    if _CAUSAL_MASK_TILES is None:
        kp = np.arange(128)[:, None]
        qc = np.arange(128)[None, :]
        _CAUSAL_MASK_TILES = np.where(qc >= kp, 0.0, NEG).astype(np.float32)
    return _CAUSAL_MASK_TILES


def _rope_tables(position_ids):
    pos = np.asarray(position_ids[0]).astype(np.float32)          # [S]
    inv_freq = (1.0 / (10000.0 ** (np.arange(0, D, 2, dtype=np.float32) / D)))
    freqs = pos[:, None] * inv_freq[None, :]                      # [S, 64]
    emb = np.concatenate([freqs, freqs], axis=1)                  # [S, 128]
    cosT = np.cos(emb).T.astype(np.float32)                       # [128, S]
    sinT = np.sin(emb).T.astype(np.float32)
    sinflipT = np.concatenate([-sinT[:64], sinT[64:]], axis=0).astype(np.float32)
    return np.ascontiguousarray(cosT), np.ascontiguousarray(sinflipT)


def kernel(hidden_states, position_ids, attention_mask, Wq, Wk, Wv, Wo):
    hidden_states = np.asarray(hidden_states)
    B = hidden_states.shape[0]
    assert hidden_states.shape == (B, S, E), hidden_states.shape
    assert B == 1

    mask = np.asarray(attention_mask, dtype=np.float32)[0, 0]
    if not mask.any():
        mode = "full"
    elif np.array_equal(mask, np.triu(np.full((S, S), NEG, dtype=np.float32), 1)):
        mode = "causal"
    else:
        mode = "general"

    if mode not in _PROGRAMS:
        _PROGRAMS[mode] = _build_program(mode)
    nc = _PROGRAMS[mode]

    hs = np.asarray(hidden_states[0], dtype=np.float32)
    # [E, S] -> group-major [NG, E, TG], bf16
    hsT = np.ascontiguousarray(
        hs.T.reshape(E, NG, TG).transpose(1, 0, 2)).astype(NPBF)
    cosT, sinflipT = _rope_tables(np.asarray(position_ids))
    # fold the 1/sqrt(D) score scaling into Wq so q and k share rope tables
    Wq = np.asarray(Wq, dtype=np.float32) * np.float32(1.0 / np.sqrt(D))
    Wk = np.asarray(Wk, dtype=np.float32)
    Wv = np.asarray(Wv, dtype=np.float32)
    Wo = np.asarray(Wo, dtype=np.float32)

    in_maps = []
    for c in range(NCORES):
        wqkv = np.concatenate([
            Wq[512 * c:512 * (c + 1), :].T,
            Wk[128 * c:128 * (c + 1), :].T,
            Wv[128 * c:128 * (c + 1), :].T,
        ], axis=1)
        m = {
            "hsT": hsT,
            "wqkvT": np.ascontiguousarray(wqkv).astype(NPBF),
            "woT": np.ascontiguousarray(Wo[:, 512 * c:512 * (c + 1)].T).astype(NPBF),
            "cosT": cosT, "sinT": sinflipT,
        }
        if mode == "causal":
            m["cmask"] = _causal_mask_tiles()
        elif mode == "general":
            m["maskT"] = np.ascontiguousarray(mask.T)
        in_maps.append(m)

    res = run_bass_kernel_spmd(nc, in_maps, core_ids=list(range(NCORES)),
                               trace=TRACE[0])
    LAST_EXEC_NS[0] = res.exec_time_ns
    LAST_RES[0] = res

    acc = np.zeros((NK, E // TG, 128, TG), dtype=np.float32)
    for c in range(NCORES):
        acc += res.results[c]["outp"].astype(np.float32)
    out = acc.transpose(0, 2, 1, 3).reshape(S, E)
    return out[None, :, :]


# revision 23
# speedup vs baseline: 1.0269x; 1.0038x over previous
"""Llama GQA attention layer (B=1, S=2048, E=4096, H=32, HKV=8, D=128) on 8
Trainium2 NeuronCores.

Sharding: tensor-parallel over heads. Core c owns Q heads 4c..4c+3 and KV head
c (KV groups stay intact), plus the matching Wo input-dim slice. Each core
computes a full [S, E] partial of the o_proj output in bf16; the host sums the
8 partials (the "all-reduce after o_proj").

All matmuls run in bf16 (1 cyc/row on the PE at 512-wide moving dim, with
automatic fast-weight-load; fp32r streams at ~1.3 cyc/row and pays 225ns
weight loads). PSUM accumulation stays fp32. l2 error budget is 2e-2; bf16
rounding of inputs/weights/probabilities lands well under 1e-2.

Per-core dataflow:
  phase A (PE-dense): per token group g (512 tokens):
    qT/kT/vT = W @ hs.T    6 psum chains x 32 E-chunks, [feat, tok] layout.
    Wq|Wk|Wv are host-packed into one [E, 768] tensor so each E-chunk is a
    single contiguous DMA and the first matmul fires ~2us after launch.
    RoPE off-PE: psum -> sbuf copy (scalar), half-swap via SBUF->SBUF DMA,
    cos/sin muls (DVE) -> qro/krope bf16. v: psum -> bf16 sbuf (scalar),
    PE-transposed to vnat [tok, d] between later groups' QKV streams.
  phase B attention, per query group G (causal: key tiles ki <= 4G+3, with
  moving-dim trimming + triangular mask add on diagonal tiles):
    scoresT[k, q] = krope_tile^T @ qro  (PSUM), exp on ScalarE -> bf16 expT
    avT[d, q]  accumulated over ki on PE (vnat stationary)
    den[h]     accumulated over ki on PE (ones[128,1] stationary, ~free
               weight load); all 4 heads share one psum bank at partition
               offsets 0/32/64/96.
    epilogue per head: DVE reciprocal on the [1, 512] den row only (full-tile
    reciprocal costs ~12 cyc/elem), K=1 ones matmul broadcasts it to 128
    partitions, DVE mul -> aoT bf16 (kept in SBUF, no DRAM spill).
  phase C o_proj: out[t, e] = sum_h aoT[:, h-tile]^T @ woT[h], 3 psum
    banks rotating, drains alternate scalar/vector, bf16 partials to DRAM.
"""

import sys
import types

if "/opt/trn_rl_repo" not in sys.path:
    sys.path.insert(0, "/opt/trn_rl_repo")

import numpy as np
import ml_dtypes

import concourse.bass as bass
import concourse.tile as tile
from concourse import bacc, mybir
from concourse.bass_utils import run_bass_kernel_spmd
from concourse.masks import make_identity

F32 = mybir.dt.float32
BF16 = mybir.dt.bfloat16
EXP = mybir.ActivationFunctionType.Exp
NPBF = ml_dtypes.bfloat16

S = 2048
E = 4096
H = 32
HKV = 8
D = 128
NCORES = 8
HL = H // NCORES          # 4 local q heads per core
TG = 512                  # token group (moving-dim tile)
NG = S // TG              # 4 token groups
NE = E // 128             # 32 contraction chunks
NK = S // 128             # 16 key tiles
FQKV = HL * D + 2 * D     # 768 packed output features per core
NEG = -1e9

TRACE = [False]
LAST_EXEC_NS = [None]
LAST_RES = [None]

_PROGRAMS = {}


def _install_ntff_hook():
    if "antenv.axon_hooks" in sys.modules:
        return
    mod = types.ModuleType("antenv.axon_hooks")
    hook = [None]
    mod.set_axon_ntff_profile_hook = lambda h: hook.__setitem__(0, h)
    mod.get_axon_ntff_profile_hook = lambda: hook[0]
    sys.modules["antenv.axon_hooks"] = mod
    try:
        from trn_agent_boot.trn_boot import _ntff_profile_via_ctypes

        mod.set_axon_ntff_profile_hook(
            _ntff_profile_via_ctypes("/opt/axon/libaxon_pjrt.so"))
    except Exception:
        pass


def set_trace(on=True):
    if on:
        _install_ntff_hook()
    TRACE[0] = on


def _build_program(mode):
    """mode: 'causal' (skip above-diagonal key tiles, trim + triangular mask
    on diagonal tiles), 'full' (no mask), 'general' (additive mask streamed
    from DRAM)."""
    nc = bacc.Bacc(trn_type="TRN2", target_bir_lowering=False, debug=False)

    # group-major hsT: [g, E, TG] so each [128, TG] chunk is contiguous
    hsT_d = nc.dram_tensor("hsT", [NG, E, TG], BF16, kind="ExternalInput").ap()
    # packed [Wq | Wk | Wv] transposed: rows are E, cols 768
    wqkv_d = nc.dram_tensor("wqkvT", [E, FQKV], BF16, kind="ExternalInput").ap()
    woT_d = nc.dram_tensor("woT", [HL * D, E], BF16, kind="ExternalInput").ap()
    cos_d = nc.dram_tensor("cosT", [D, S], F32, kind="ExternalInput").ap()
    sin_d = nc.dram_tensor("sinT", [D, S], F32, kind="ExternalInput").ap()
    if mode == "causal":
        cmask_d = nc.dram_tensor("cmask", [128, 128], F32,
                                 kind="ExternalInput").ap()
    elif mode == "general":
        maskT_d = nc.dram_tensor("maskT", [S, S], F32, kind="ExternalInput").ap()
    # tile-major output: [ti, eg, 128, TG] so each store is contiguous
    outp_d = nc.dram_tensor("outp", [NK, E // TG, 128, TG], BF16,
                            kind="ExternalOutput").ap()

    with tile.TileContext(nc) as tc:
        with tc.tile_pool(name="const", bufs=1) as cpool, \
             tc.tile_pool(name="persist", bufs=1) as pp, \
             tc.tile_pool(name="wqkv", bufs=1) as wp, \
             tc.tile_pool(name="cs", bufs=1) as csp, \
             tc.tile_pool(name="hst", bufs=12) as hp, \
             tc.tile_pool(name="rope", bufs=1) as rp, \
             tc.tile_pool(name="attn", bufs=1) as ap_, \
             tc.tile_pool(name="outb", bufs=1) as obp, \
             tc.tile_pool(name="ps", bufs=1, space="PSUM") as ps:

            # ---- constants ----
            identf = cpool.tile([128, 128], F32)
            make_identity(nc, identf)
            ident = cpool.tile([128, 128], BF16)
            nc.vector.tensor_copy(ident, identf)
            onesf = cpool.tile([128, 32], F32)
            nc.vector.memset(onesf, 1.0)
            ones_col = cpool.tile([128, 32], BF16)
            nc.vector.tensor_copy(ones_col, onesf)
            # ones rows at partition bases 0 and 32 (matmul operands must
            # share a 32-aligned base partition with the den rows they read)
            onesrf = cpool.tile([64, 128], F32)
            nc.vector.memset(onesrf, 1.0)
            ones_rows = cpool.tile([64, 128], BF16)
            nc.vector.tensor_copy(ones_rows, onesrf)
            if mode == "causal":
                cmask = cpool.tile([128, 128], F32)

            # ---- persistent activations ----
            krope = pp.tile([128, S], BF16)               # [d, tok]
            vnat = pp.tile([128, NK, 128], BF16)          # [tok%128, ktile, d]
            ao = pp.tile([128, HL, S], BF16)              # [d, head, tok]
            qro = pp.tile([128, NG, HL, TG], BF16)        # [d, g, head, tok]

            # ---- weights: chunk loads interleaved with group-0 hsT so the
            # first QKV matmul fires almost immediately ----
            w_sb = wp.tile([128, NE, FQKV], BF16)
            wo_sb = wp.tile([128, HL, E], BF16)
            cos_sb = csp.tile([128, S], F32)
            sin_sb = csp.tile([128, S], F32)

            # weight chunks batched x4 on the scalar queue, group-0 hsT on the
            # sync queue: both pipelines issue in parallel so the first
            # matmul fires as early as possible (each dma_start costs ~0.6us
            # of descriptor generation on its issuing queue)
            wqkv_r = wqkv_d.rearrange("(ne p) f -> p ne f", p=128)
            hst0 = []
            for e in range(NE):
                if e % 4 == 0:
                    nc.scalar.dma_start(out=w_sb[:, e:e + 4, :],
                                        in_=wqkv_r[:, e:e + 4, :])
                hst = hp.tile([128, TG], BF16, tag="hst")
                nc.sync.dma_start(
                    out=hst, in_=hsT_d[0, 128 * e:128 * (e + 1), :])
                hst0.append(hst)
            # cos/sin (2MB) + cmask load after the weights; not needed
            # until rope(0)/attention
            nc.scalar.dma_start(out=cos_sb, in_=cos_d)
            nc.scalar.dma_start(out=sin_sb, in_=sin_d)
            if mode == "causal":
                nc.scalar.dma_start(out=cmask, in_=cmask_d)

            # ================= phase A: QKV projection + RoPE =================
            def emit_qkv(g):
                q_ps = [ps.tile([128, TG], F32, tag=f"A{f}", name=f"q_ps{f}")
                        for f in range(HL)]
                k_ps = ps.tile([128, TG], F32, tag="A4", name="k_ps")
                v_ps = ps.tile([128, TG], F32, tag="A5", name="v_ps")
                for e in range(NE):
                    if g == 0:
                        hst = hst0[e]
                    else:
                        hst = hp.tile([128, TG], BF16, tag="hst")
                        nc.sync.dma_start(
                            out=hst, in_=hsT_d[g, 128 * e:128 * (e + 1), :])
                    st, sp = (e == 0), (e == NE - 1)
                    for f in range(HL):
                        nc.tensor.matmul(
                            q_ps[f], w_sb[:, e, 128 * f:128 * (f + 1)],
                            hst, start=st, stop=sp)
                    nc.tensor.matmul(k_ps, w_sb[:, e, 512:640], hst,
                                     start=st, stop=sp)
                    nc.tensor.matmul(v_ps, w_sb[:, e, 640:768], hst,
                                     start=st, stop=sp)
                return q_ps, k_ps, v_ps

            def emit_rope(g, q_ps, k_ps, v_ps):
                t0 = g * TG
                cs = cos_sb[:, t0:t0 + TG]
                sn = sin_sb[:, t0:t0 + TG]
                pairs = [(q_ps[f], qro[:, g, f, :]) for f in range(HL)]
                kpair = (k_ps, krope[:, t0:t0 + TG])
                # last group: drain k first so attention's first score matmul
                # (s-bank reuse of the k psum tag) unblocks earliest
                pairs = [kpair] + pairs if g == NG - 1 else pairs + [kpair]
                # drain ALL six psum banks first, copies split across scalar
                # and vector so the next group's QKV matmuls unblock in ~1us;
                # the rope math below then reads only SBUF
                vs = rp.tile([128, TG], BF16, tag="vs", bufs=2)
                if g == NG - 1:
                    nc.scalar.copy(out=vs, in_=v_ps)
                xss = []
                for i, (x_ps, _) in enumerate(pairs):
                    xs = rp.tile([128, TG], F32, tag="xs", bufs=6)
                    # last group: keep the scalar queue clear for attention's
                    # exps -- only k drains there, the q heads go to vector
                    if (i == 0) if g == NG - 1 else (i % 2 == 0):
                        nc.scalar.copy(out=xs, in_=x_ps)
                    else:
                        nc.vector.tensor_copy(xs, x_ps)
                    xss.append(xs)
                if g != NG - 1:
                    nc.scalar.copy(out=vs, in_=v_ps)

                def math():
                    for xs, (_, out_ap) in zip(xss, pairs):
                        swp = rp.tile([128, TG], F32, tag="swp", bufs=3)
                        nc.gpsimd.dma_start(out=swp[0:64, :], in_=xs[64:128, :])
                        nc.gpsimd.dma_start(out=swp[64:128, :], in_=xs[0:64, :])
                        p1 = rp.tile([128, TG], F32, tag="p1", bufs=2)
                        nc.vector.tensor_mul(p1, xs, cs)
                        nc.vector.tensor_mul(swp, swp, sn)
                        nc.vector.tensor_add(out_ap, p1, swp)
                # group 3's rope outputs are only read by attention(3): defer
                # its math emission behind attention(1) so groups 0/1 (short
                # head streams, every tile diagonal-masked) don't queue their
                # cmask adds and reciprocals behind it on the vector engine
                if g == NG - 1:
                    return vs, math
                math()
                return vs, None

            def emit_vtr(g, vs):
                for j in range(4):
                    tr = ps.tile([128, 128], BF16, tag="A6", name="tr_ps")
                    nc.tensor.transpose(tr, vs[:, 128 * j:128 * (j + 1)], ident)
                    nc.vector.tensor_copy(vnat[:, 4 * g + j, :], tr)

            vs_pend = []
            rope3_math = [None]
            for g in range(NG):
                qkv = emit_qkv(g)
                if vs_pend:
                    emit_vtr(*vs_pend.pop())
                vs, m = emit_rope(g, *qkv)
                rope3_math[0] = m
                vs_pend.append((g, vs))
            emit_vtr(*vs_pend.pop())

            # wo loads issue from the (otherwise idle) gpsimd queue during
            # attention
            woT_r = woT_d.rearrange("(h p) e -> p h e", p=128)
            for eg in range(E // TG):
                nc.gpsimd.dma_start(
                    out=wo_sb[:, :, TG * eg:TG * (eg + 1)],
                    in_=woT_r[:, :, TG * eg:TG * (eg + 1)])

            # ================= phase B: attention =================
            # Head-major: each head's full key sweep completes before the next
            # head begins, so the per-head softmax epilogue (3.3us flat DVE
            # reciprocal) overlaps the NEXT head's matmul stream instead of
            # four reciprocals serializing at the group boundary. Banks:
            # av alternates A0/A1, scores alternate A4/A5, each head owns a
            # private den bank (A2/A3/A6/A7) whose tag is reused for its own
            # broadcast matmul. Epilogues defer by at most one head: the
            # flush invariant keeps every bank's previous reader emitted
            # before its next writer (PE-queue deadlock freedom).
            deferred = []

            def emit_attn(G):
                nk = 4 * G + 4 if mode == "causal" else NK
                t0 = G * TG
                for h in range(HL):
                    av = ps.tile([128, TG], F32, tag=["A0", "A1"][h % 2],
                                 name=f"av{h}")
                    den = ps.tile([128, TG], F32, name="den",
                                  tag=["A2", "A3", "A6", "A7"][h])
                    pend = []

                    def drain(item, av=av, den=den, nk=nk):
                        ki, c0, ex = item
                        nc.tensor.matmul(den[0:32, c0:], ones_col,
                                         ex[:, c0:],
                                         start=(ki == 0), stop=(ki == nk - 1),
                                         skip_group_check=True)
                        nc.tensor.matmul(av[:, c0:], vnat[:, ki, :],
                                         ex[:, c0:], start=(ki == 0),
                                         stop=(ki == nk - 1),
                                         skip_group_check=True)

                    for ki in range(nk):
                        c0 = max(0, 128 * ki - TG * G) if mode == "causal" else 0
                        s = ps.tile([128, TG], F32, name="s_ps",
                                    tag=["A4", "A5"][ki % 2])
                        nc.tensor.matmul(s[:, c0:],
                                         krope[:, 128 * ki:128 * (ki + 1)],
                                         qro[:, G, h, c0:],
                                         start=True, stop=True)
                        if mode == "causal" and ki >= 4 * G:
                            nc.vector.tensor_add(s[:, c0:c0 + 128],
                                                 s[:, c0:c0 + 128], cmask)
                        elif mode == "general":
                            mt = ap_.tile([128, TG], F32, tag="mt", bufs=4)
                            nc.sync.dma_start(
                                out=mt, in_=maskT_d[128 * ki:128 * (ki + 1),
                                                    TG * G:TG * (G + 1)])
                            nc.vector.tensor_add(s, s, mt)
                        ex = ap_.tile([128, TG], BF16, tag="ex", bufs=10)
                        nc.scalar.activation(out=ex[:, c0:], in_=s[:, c0:],
                                             func=EXP)
                        pend.append((ki, c0, ex))
                        # consume the previous heads' epilogues as late as
                        # the bank-reuse deadlock rule allows (before this
                        # head's first av/den drain at ki==3), giving their
                        # reciprocals maximum cover
                        if ki == 2:
                            while len(deferred) > 1:
                                deferred.pop(0)()
                        elif ki >= 6 and deferred:
                            deferred.pop(0)()
                        while len(pend) > 3:
                            drain(pend.pop(0))
                    while pend:
                        drain(pend.pop(0))
                    # reciprocal fires now, hidden under the next head's
                    # matmuls; the psum-side epilogue is deferred
                    rcb = ap_.tile([64, TG], BF16, tag="rcb", bufs=3)
                    with nc.allow_low_precision(reason="softmax recip"):
                        nc.vector.reciprocal(rcb[0:1, :], den[0:1, :])
                    bc = ps.tile([128, TG], F32, name="bc_ps",
                                 tag=["A2", "A3", "A6", "A7"][h])

                    def mk(h=h, rcb=rcb, bc=bc, avh=av, t0=t0):
                        def emit_epi():
                            nc.tensor.matmul(bc, ones_rows[0:1, :],
                                             rcb[0:1, :], start=True,
                                             stop=True)
                            bcs = ap_.tile([128, TG], BF16, tag="bcs", bufs=2)
                            nc.vector.tensor_copy(bcs, bc)
                            nc.vector.tensor_mul(ao[:, h, t0:t0 + TG], avh,
                                                 bcs)
                        return emit_epi
                    deferred.append(mk())

            for G in range(NG):
                emit_attn(G)
                if G == 1 and rope3_math[0] is not None:
                    rope3_math[0]()

            # ================= phase C: o_proj =================
            # one deferred epilogue (group 3 head 3) remains; flush it after
            # the first o_proj psum block (tag A0) so its reciprocal gets
            # cover, before the A1 block that reuses head 3's av bank
            for ti in range(NK):
                for eg in range(E // TG):
                    o_ps = ps.tile([128, TG], F32, name="o_ps",
                                   tag=["A0", "A4", "A1", "A5"][(ti * 8 + eg) % 4])
                    for h in range(HL):
                        nc.tensor.matmul(
                            o_ps, ao[:, h, 128 * ti:128 * (ti + 1)],
                            wo_sb[:, h, TG * eg:TG * (eg + 1)],
                            start=(h == 0), stop=(h == HL - 1))
                    while deferred:
                        deferred.pop(0)()
                    ob = obp.tile([128, TG], BF16, tag="ob", bufs=6)
                    if eg % 2 == 0:
                        nc.scalar.copy(out=ob, in_=o_ps)
                    else:
                        nc.vector.tensor_copy(ob, o_ps)
                    if eg % 2 == 0:
                        nc.gpsimd.dma_start(out=outp_d[ti, eg], in_=ob)
                    else:
                        nc.sync.dma_start(out=outp_d[ti, eg], in_=ob)

    nc.compile()
    return nc


_CAUSAL_MASK_TILES = None


def _causal_mask_tiles():
    global _CAUSAL_MASK_TILES
    if _CAUSAL_MASK_TILES is None:
        kp = np.arange(128)[:, None]
        qc = np.arange(128)[None, :]
        _CAUSAL_MASK_TILES = np.where(qc >= kp, 0.0, NEG).astype(np.float32)
    return _CAUSAL_MASK_TILES


def _rope_tables(position_ids):
    pos = np.asarray(position_ids[0]).astype(np.float32)          # [S]
    inv_freq = (1.0 / (10000.0 ** (np.arange(0, D, 2, dtype=np.float32) / D)))
    freqs = pos[:, None] * inv_freq[None, :]                      # [S, 64]
    emb = np.concatenate([freqs, freqs], axis=1)                  # [S, 128]
    cosT = np.cos(emb).T.astype(np.float32)                       # [128, S]
    sinT = np.sin(emb).T.astype(np.float32)
    sinflipT = np.concatenate([-sinT[:64], sinT[64:]], axis=0).astype(np.float32)
    return np.ascontiguousarray(cosT), np.ascontiguousarray(sinflipT)


def kernel(hidden_states, position_ids, attention_mask, Wq, Wk, Wv, Wo):
    hidden_states = np.asarray(hidden_states)
    B = hidden_states.shape[0]
    assert hidden_states.shape == (B, S, E), hidden_states.shape
    assert B == 1

    mask = np.asarray(attention_mask, dtype=np.float32)[0, 0]
    if not mask.any():
        mode = "full"
    elif np.array_equal(mask, np.triu(np.full((S, S), NEG, dtype=np.float32), 1)):
        mode = "causal"
    else:
        mode = "general"

    if mode not in _PROGRAMS:
        _PROGRAMS[mode] = _build_program(mode)
    nc = _PROGRAMS[mode]

    hs = np.asarray(hidden_states[0], dtype=np.float32)
    # [E, S] -> group-major [NG, E, TG], bf16
    hsT = np.ascontiguousarray(
        hs.T.reshape(E, NG, TG).transpose(1, 0, 2)).astype(NPBF)
    cosT, sinflipT = _rope_tables(np.asarray(position_ids))
    # fold the 1/sqrt(D) score scaling into Wq so q and k share rope tables
    Wq = np.asarray(Wq, dtype=np.float32) * np.float32(1.0 / np.sqrt(D))
    Wk = np.asarray(Wk, dtype=np.float32)
    Wv = np.asarray(Wv, dtype=np.float32)
    Wo = np.asarray(Wo, dtype=np.float32)

    in_maps = []
    for c in range(NCORES):
        wqkv = np.concatenate([
            Wq[512 * c:512 * (c + 1), :].T,
            Wk[128 * c:128 * (c + 1), :].T,
            Wv[128 * c:128 * (c + 1), :].T,
        ], axis=1)
        m = {
            "hsT": hsT,
            "wqkvT": np.ascontiguousarray(wqkv).astype(NPBF),
            "woT": np.ascontiguousarray(Wo[:, 512 * c:512 * (c + 1)].T).astype(NPBF),
            "cosT": cosT, "sinT": sinflipT,
        }
        if mode == "causal":
            m["cmask"] = _causal_mask_tiles()
        elif mode == "general":
            m["maskT"] = np.ascontiguousarray(mask.T)
        in_maps.append(m)

    res = run_bass_kernel_spmd(nc, in_maps, core_ids=list(range(NCORES)),
                               trace=TRACE[0])
    LAST_EXEC_NS[0] = res.exec_time_ns
    LAST_RES[0] = res

    acc = np.zeros((NK, E // TG, 128, TG), dtype=np.float32)
    for c in range(NCORES):
        acc += res.results[c]["outp"].astype(np.float32)
    out = acc.transpose(0, 2, 1, 3).reshape(S, E)
    return out[None, :, :]
